# revision 20
# baseline (speedup 1.0000x reference)
"""Chamfer-distance kernel for 8 Trainium2 NeuronCores (Bass/Tile).

Problem: gts [8, 8192, 3] f32, preds [8, 8192, 3] f32 ->
         scalar chamfer distance (pytorch3d convention: squared L2,
         mean over points, mean over batch, sum of both directions).

Sharding: one batch element per NeuronCore (B == n_cores == 8).

Per-core algorithm (min convention):
  - d2[n, m] = |x_n|^2 + |y_m|^2 - 2 x.y is produced by the PE as a single
    K=24 bf16 matmul per [128 x 512] tile: x/y coordinates are split into
    3-term bf16 hi/mid/lo components (fp32-class accuracy); |x|^2 / |y|^2
    enter as extra contraction rows against a ones-row.  The packed
    operands are replicated at partition offsets 0/32/64/96 so consecutive
    matmuls use different PE row-groups (hides LDWEIGHTS, enables
    concurrent sub-array execution).
  - ScalarE copies each PSUM group [128, 2048] to a bf16 row buffer.
  - VectorE keeps a running elementwise column-min over n-tiles
    (colmin [128, 8192] bf16) and computes row-mins by a halving
    bf16 tensor-tensor min fold over each row block.
  - Host: means + relu clamp + batch mean (epilogue on tiny outputs).
"""

import sys

sys.path.insert(0, "/opt/trn_rl_repo")

import numpy as np
import ml_dtypes

import concourse.bacc as bacc
import concourse.mybir as mybir
import concourse.tile as tile
from concourse.bass_utils import run_bass_kernel_spmd

BF16 = ml_dtypes.bfloat16
POS_BIG = 3.0e38

B = 8
N_PTS = 8192
M_PTS = 8192
K_ROWS = 24
ROT = 4
H_ROWS = 32 * (ROT - 1) + K_ROWS
GROUP = 2048


def _split3(v):
    """float64 vector -> three bf16-representable float64 components."""
    a0 = v.astype(BF16).astype(np.float64)
    a1 = (v - a0).astype(BF16).astype(np.float64)
    a2 = (v - a0 - a1).astype(BF16).astype(np.float64)
    return a0, a1, a2


def _host_pack(x, y):
    """x [N,3] f32, y [M,3] f32 -> xp [H,N] bf16, yp [H,M] bf16 such that
    sum_k xp[k,n]*yp[k,m] = d2(x_n, y_m), replicated at 4 row offsets."""
    N, M = x.shape[0], y.shape[0]
    xd = x.astype(np.float64)
    yd = y.astype(np.float64)
    ax = [_split3(xd[:, d]) for d in range(3)]
    by = [_split3(yd[:, d]) for d in range(3)]
    x2 = _split3((xd * xd).sum(1))
    y2 = _split3((yd * yd).sum(1))

    xp = np.zeros((K_ROWS, N), dtype=BF16)
    yp = np.zeros((K_ROWS, M), dtype=BF16)
    r = 0
    for (i, j) in [(0, 0), (0, 1), (1, 0), (1, 1), (0, 2), (2, 0)]:
        for d in range(3):
            xp[r] = (-2.0 * ax[d][i]).astype(BF16)
            yp[r] = by[d][j].astype(BF16)
            r += 1
    for k in range(3):
        xp[r] = x2[k].astype(BF16)
        yp[r] = np.ones(M, dtype=BF16)
        r += 1
    for k in range(3):
        xp[r] = np.ones(N, dtype=BF16)
        yp[r] = y2[k].astype(BF16)
        r += 1
    assert r == K_ROWS

    xr = np.zeros((H_ROWS, N), dtype=BF16)
    yr = np.zeros((H_ROWS, M), dtype=BF16)
    for q in range(ROT):
        xr[32 * q:32 * q + K_ROWS] = xp
        yr[32 * q:32 * q + K_ROWS] = yp
    return xr, yr


def _build_nc(reps=1):
    f32 = mybir.dt.float32
    bf16 = mybir.dt.bfloat16
    MIN = mybir.AluOpType.min
    nc = bacc.Bacc()
    n_tiles = N_PTS // 128
    n_groups = M_PTS // GROUP
    cpg = GROUP // 512

    xp = nc.declare_dram_parameter("xp", [H_ROWS, N_PTS], bf16, isOutput=False)
    yp = nc.declare_dram_parameter("yp", [H_ROWS, M_PTS], bf16, isOutput=False)
    # rowout holds each tile's rows folded down to width 2048; the host does
    # the final 2048->1 min (cheaper than more 2x-mode fold levels on DVE;
    # the extra DMA rides idle HBM write bandwidth).
    rowout = nc.declare_dram_parameter("rowout", [128, n_tiles * 2048], bf16,
                                       isOutput=True)
    colout = nc.declare_dram_parameter("colout", [128, M_PTS], bf16, isOutput=True)

    with tile.TileContext(nc) as tc:
        with (
            tc.tile_pool(name="inputs", bufs=1) as inp,
            tc.tile_pool(name="acc", bufs=1) as acc,
            tc.tile_pool(name="cp", bufs=3) as cpp,
            tc.tile_pool(name="ps", bufs=2, space="PSUM") as psp,
        ):
            xs = inp.tile([H_ROWS, N_PTS], bf16)
            ys = inp.tile([H_ROWS, M_PTS], bf16)
            nc.sync.dma_start(xs[:], xp[:])
            nc.sync.dma_start(ys[:], yp[:])

            colmin = acc.tile([128, M_PTS], bf16)

            def body():
                wide_ref = [None]
                for t in range(n_tiles):
                    h = t % 2
                    if h == 0:
                        wide_ref[0] = cpp.tile([128, 2 * M_PTS], bf16,
                                               name="wide")
                    wide = wide_ref[0]
                    base = h * M_PTS
                    for g in range(n_groups):
                        ps = psp.tile([128, GROUP], f32, name="ps")
                        for j in range(cpg):
                            c = g * cpg + j
                            lo = 32 * (c % ROT)
                            nc.tensor.matmul(
                                ps[:, j * 512:(j + 1) * 512],
                                xs[lo:lo + K_ROWS, t * 128:(t + 1) * 128],
                                ys[lo:lo + K_ROWS, c * 512:(c + 1) * 512],
                                start=True,
                                stop=True,
                                tile_position=(lo, 0),
                            )
                        cs = wide[:, base + g * GROUP:base + (g + 1) * GROUP]
                        nc.scalar.copy(cs, ps[:])
                        cm = colmin[:, g * GROUP:(g + 1) * GROUP]
                        if t == 0:
                            pass  # colmin seeded at t == 1 from both rows
                        elif t == 1:
                            nc.vector.tensor_tensor(
                                cm, cs, wide[:, g * GROUP:(g + 1) * GROUP],
                                op=MIN)
                        else:
                            nc.vector.tensor_tensor(cm, cs, cm, op=MIN)
                    if h == 1:
                        # pair-batched halving min-fold on both rows at once
                        view = wide[:].rearrange("p (a b) -> p a b", a=2)
                        w = M_PTS // 2
                        while w >= 2048:
                            nc.vector.tensor_tensor(
                                view[:, :, :w], view[:, :, :w],
                                view[:, :, w:2 * w], op=MIN)
                            w //= 2
                        nc.gpsimd.dma_start(
                            rowout[:, (t - 1) * 2048:(t + 1) * 2048],
                            view[:, :, :2048])

            if reps == 1:
                body()
            else:
                with tc.For_i(0, reps, 1):
                    body()

            nc.sync.dma_start(colout[:], colmin[:])
    nc.compile()
    return nc


_CACHED_NC = None


def _get_nc():
    global _CACHED_NC
    if _CACHED_NC is None:
        _CACHED_NC = _build_nc(reps=1)
    return _CACHED_NC


def kernel(gts, preds):
    gts = np.asarray(gts, dtype=np.float32)
    preds = np.asarray(preds, dtype=np.float32)
    assert gts.shape == (B, N_PTS, 3) and preds.shape == (B, M_PTS, 3), (
        gts.shape, preds.shape)

    nc = _get_nc()
    in_maps = []
    for b in range(B):
        xp, yp = _host_pack(gts[b], preds[b])
        in_maps.append({"xp": xp, "yp": yp})

    res = run_bass_kernel_spmd(nc, in_maps, list(range(B)))

    n_tiles = N_PTS // 128
    total = 0.0
    for b in range(B):
        rf = res.results[b]["rowout"].astype(np.float32)     # [128, 64*2048]
        rmin = rf.reshape(128, n_tiles, 2048).min(axis=2).astype(np.float64)
        cmin = res.results[b]["colout"].astype(np.float64)   # [128, M]
        sx = np.maximum(rmin, 0.0).sum()
        sy = np.maximum(cmin.min(axis=0), 0.0).sum()
        total += sx / N_PTS + sy / M_PTS
    return np.float32(total / B)



# revision 23
# speedup vs baseline: 1.3748x; 1.3748x over previous
"""Chamfer-distance kernel for 8 Trainium2 NeuronCores (Bass/Tile).

Problem: gts [8, 8192, 3] f32, preds [8, 8192, 3] f32 ->
         scalar chamfer distance (pytorch3d convention: squared L2,
         mean over points, mean over batch, sum of both directions).

Sharding: one batch element per NeuronCore (B == n_cores == 8).

Per-core algorithm (min convention):
  - d2[n, m] = |x_n|^2 + |y_m|^2 - 2 x.y is produced by the PE as a single
    K=24 bf16 matmul per [128 x 512] tile: x/y coordinates are split into
    3-term bf16 hi/mid/lo components (fp32-class accuracy); |x|^2 / |y|^2
    enter as extra contraction rows against a ones-row.  The packed
    operands are replicated at partition offsets 0/32/64/96 so consecutive
    matmuls use different PE row-groups (hides LDWEIGHTS, enables
    concurrent sub-array execution).
  - ScalarE copies each PSUM group [128, 2048] to a bf16 row buffer.
    Two consecutive n-tiles share one double-wide [128, 2*8192] buffer.
  - VectorE keeps a running elementwise column-min over n-tiles
    (colmin [128, 8192] bf16; seeded at t==1 from min(row0, row1) so the
    t==0 pass is skipped) and row-direction mins as pair-batched halving
    tensor-tensor min folds (3D [128, 2, w] APs fold both tiles of a pair
    per instruction, 2x bf16 mode) down to width 512 only.
  - The [128, 2, 512] fold partials are DMA'd out per pair on the
    otherwise-idle gpsimd (SWDGE) queue; the host finishes the 512->1
    row min.  This keeps the 1x-mode tensor_reduce off VectorE, which is
    the critical engine (~100% busy; ScalarE ~85%, PE ~45%).
  - Host: final column min over partitions, 512->1 row mins, relu clamp,
    means (epilogue on tiny outputs).
"""

import sys

sys.path.insert(0, "/opt/trn_rl_repo")

import numpy as np
import ml_dtypes

import concourse.bacc as bacc
import concourse.mybir as mybir
import concourse.tile as tile
from concourse.bass_utils import run_bass_kernel_spmd

BF16 = ml_dtypes.bfloat16
POS_BIG = 3.0e38

B = 8
N_PTS = 8192
M_PTS = 8192
K_ROWS = 24
ROT = 4
H_ROWS = 32 * (ROT - 1) + K_ROWS
GROUP = 2048


def _split3(v):
    """float64 vector -> three bf16-representable float64 components."""
    a0 = v.astype(BF16).astype(np.float64)
    a1 = (v - a0).astype(BF16).astype(np.float64)
    a2 = (v - a0 - a1).astype(BF16).astype(np.float64)
    return a0, a1, a2


def _host_pack(x, y):
    """x [N,3] f32, y [M,3] f32 -> xp [H,N] bf16, yp [H,M] bf16 such that
    sum_k xp[k,n]*yp[k,m] = d2(x_n, y_m), replicated at 4 row offsets."""
    N, M = x.shape[0], y.shape[0]
    xd = x.astype(np.float64)
    yd = y.astype(np.float64)
    ax = [_split3(xd[:, d]) for d in range(3)]
    by = [_split3(yd[:, d]) for d in range(3)]
    x2 = _split3((xd * xd).sum(1))
    y2 = _split3((yd * yd).sum(1))

    xp = np.zeros((K_ROWS, N), dtype=BF16)
    yp = np.zeros((K_ROWS, M), dtype=BF16)
    r = 0
    for (i, j) in [(0, 0), (0, 1), (1, 0), (1, 1), (0, 2), (2, 0)]:
        for d in range(3):
            xp[r] = (-2.0 * ax[d][i]).astype(BF16)
            yp[r] = by[d][j].astype(BF16)
            r += 1
    for k in range(3):
        xp[r] = x2[k].astype(BF16)
        yp[r] = np.ones(M, dtype=BF16)
        r += 1
    for k in range(3):
        xp[r] = np.ones(N, dtype=BF16)
        yp[r] = y2[k].astype(BF16)
        r += 1
    assert r == K_ROWS

    xr = np.zeros((H_ROWS, N), dtype=BF16)
    yr = np.zeros((H_ROWS, M), dtype=BF16)
    for q in range(ROT):
        xr[32 * q:32 * q + K_ROWS] = xp
        yr[32 * q:32 * q + K_ROWS] = yp
    return xr, yr


def _build_nc(reps=1):
    f32 = mybir.dt.float32
    bf16 = mybir.dt.bfloat16
    MIN = mybir.AluOpType.min
    nc = bacc.Bacc()
    n_tiles = N_PTS // 128
    n_groups = M_PTS // GROUP
    cpg = GROUP // 512

    xp = nc.declare_dram_parameter("xp", [H_ROWS, N_PTS], bf16, isOutput=False)
    yp = nc.declare_dram_parameter("yp", [H_ROWS, M_PTS], bf16, isOutput=False)
    # rowout holds each tile's rows folded down to width 1024; the host does
    # the final 1024->1 min (cheaper than more fold levels on the critical
    # VectorE; 16MB/core of DMA rides at ~44GB/s, well under the kernel span).
    rowout = nc.declare_dram_parameter("rowout", [128, n_tiles * 1024], bf16,
                                       isOutput=True)
    colout = nc.declare_dram_parameter("colout", [128, M_PTS], bf16, isOutput=True)

    with tile.TileContext(nc) as tc:
        with (
            tc.tile_pool(name="inputs", bufs=1) as inp,
            tc.tile_pool(name="acc", bufs=1) as acc,
            tc.tile_pool(name="cp", bufs=3) as cpp,
            tc.tile_pool(name="ps", bufs=2, space="PSUM") as psp,
        ):
            xs = inp.tile([H_ROWS, N_PTS], bf16)
            ys = inp.tile([H_ROWS, M_PTS], bf16)
            nc.sync.dma_start(xs[:], xp[:])
            nc.sync.dma_start(ys[:], yp[:])

            colmin = acc.tile([128, M_PTS], bf16)

            def body():
                wide_ref = [None]
                for t in range(n_tiles):
                    h = t % 2
                    if h == 0:
                        wide_ref[0] = cpp.tile([128, 2 * M_PTS], bf16,
                                               name="wide")
                    wide = wide_ref[0]
                    base = h * M_PTS
                    for g in range(n_groups):
                        ps = psp.tile([128, GROUP], f32, name="ps")
                        for j in range(cpg):
                            c = g * cpg + j
                            lo = 32 * (c % ROT)
                            nc.tensor.matmul(
                                ps[:, j * 512:(j + 1) * 512],
                                xs[lo:lo + K_ROWS, t * 128:(t + 1) * 128],
                                ys[lo:lo + K_ROWS, c * 512:(c + 1) * 512],
                                start=True,
                                stop=True,
                                tile_position=(lo, 0),
                            )
                        cs = wide[:, base + g * GROUP:base + (g + 1) * GROUP]
                        nc.scalar.copy(cs, ps[:])
                        cm = colmin[:, g * GROUP:(g + 1) * GROUP]
                        if t == 0:
                            pass  # colmin seeded at t == 1 from both rows
                        elif t == 1:
                            nc.vector.tensor_tensor(
                                cm, cs, wide[:, g * GROUP:(g + 1) * GROUP],
                                op=MIN)
                        else:
                            nc.vector.tensor_tensor(cm, cs, cm, op=MIN)
                    if h == 1:
                        # pair-batched halving min-fold on both rows at once
                        view = wide[:].rearrange("p (a b) -> p a b", a=2)
                        w = M_PTS // 2
                        while w >= 1024:
                            nc.vector.tensor_tensor(
                                view[:, :, :w], view[:, :, :w],
                                view[:, :, w:2 * w], op=MIN)
                            w //= 2
                        # alternate the two idle DMA queues (Pool SWDGE / SP)
                        dma_eng = nc.gpsimd if (t // 2) % 2 == 0 else nc.sync
                        dma_eng.dma_start(
                            rowout[:, (t - 1) * 1024:(t + 1) * 1024],
                            view[:, :, :1024])

            if reps == 1:
                body()
            else:
                with tc.For_i(0, reps, 1):
                    body()

            nc.sync.dma_start(colout[:], colmin[:])
    nc.compile()
    return nc


_CACHED_NC = None


def _get_nc():
    global _CACHED_NC
    if _CACHED_NC is None:
        _CACHED_NC = _build_nc(reps=1)
    return _CACHED_NC


def kernel(gts, preds):
    gts = np.asarray(gts, dtype=np.float32)
    preds = np.asarray(preds, dtype=np.float32)
    assert gts.shape == (B, N_PTS, 3) and preds.shape == (B, M_PTS, 3), (
        gts.shape, preds.shape)

    nc = _get_nc()
    in_maps = []
    for b in range(B):
        xp, yp = _host_pack(gts[b], preds[b])
        in_maps.append({"xp": xp, "yp": yp})

    res = run_bass_kernel_spmd(nc, in_maps, list(range(B)))

    n_tiles = N_PTS // 128
    total = 0.0
    for b in range(B):
        rf = res.results[b]["rowout"].astype(np.float32)     # [128, 64*1024]
        rmin = rf.reshape(128, n_tiles, 1024).min(axis=2).astype(np.float64)
        cmin = res.results[b]["colout"].astype(np.float64)   # [128, M]
        sx = np.maximum(rmin, 0.0).sum()
        sy = np.maximum(cmin.min(axis=0), 0.0).sum()
        total += sx / N_PTS + sy / M_PTS
    return np.float32(total / B)

